# revision 18
# baseline (speedup 1.0000x reference)
"""PointWarping v4: fp16 score selection on device + exact host re-rank.

Device per core: augmented matmul scores (f32 PSUM) are cast to fp16 on
the PSUM->SBUF copy; DVE max / max_index run at 2x 16-bit throughput and
return the top-8 candidate values+indices per query.  Host re-ranks the
8 candidates with exact f32 distances (reference formula), computes the
weights, gathers neighbor flows and warps.  Queries where the fp16
3rd==8th value ties (candidate set not provably complete) or duplicate
indices appear are recomputed exactly on host (rare).

Perf notes (axon-tunneled cores: ~80ms RTT, ~170MB/s, ~6ms/exec launch):
- the PJRT shard_map executable is built once and cached
- no donated zero output buffers (kernel writes every output element),
  so nothing but the real inputs is uploaded per call
- all D2H copies start async so both outputs fetch in one roundtrip
- vallo ships only the rank-2 and rank-7 values (the tie flag inputs)
- host combine is vectorized over all 8 cores; 16-byte-row gathers go
  through a complex128 view (single-element fancy indexing)
"""

import numpy as np

B, C, N = 4, 3, 8192
NQ = 4096
NT = 32
EPS = 1e-10
CLAMP = 10.0

_CACHE = {}


def _build():
    if "nc" in _CACHE:
        return _CACHE["nc"]

    from contextlib import ExitStack
    from concourse import bacc, bass, tile
    from concourse import mybir

    nc = bacc.Bacc("TRN2", target_bir_lowering=False, debug=False,
                   enable_asserts=True, num_devices=1)
    f32 = mybir.dt.float32
    f32r = mybir.dt.float32r
    f16 = mybir.dt.float16
    i16 = mybir.dt.int16
    u32 = mybir.dt.uint32
    ADD = mybir.AluOpType.add
    MULT = mybir.AluOpType.mult

    q2 = nc.dram_tensor("q2", [3, NQ], f16, kind="ExternalInput").ap()
    kb = nc.dram_tensor("kb", [32, 768], f16, kind="ExternalInput").ap()
    vallo = nc.dram_tensor("vallo", [128, 2 * NT], f16,
                           kind="ExternalOutput").ap()
    gidxo = nc.dram_tensor("gidxo", [128, 8 * NT], i16,
                           kind="ExternalOutput").ap()

    with tile.TileContext(nc) as tc, ExitStack() as ctx:
        cp = ctx.enter_context(tc.tile_pool(name="persist", bufs=1))
        spool = ctx.enter_context(tc.tile_pool(name="scores", bufs=2))
        ppool = ctx.enter_context(tc.tile_pool(name="ps", bufs=2, space="PSUM"))
        tp = ctx.enter_context(tc.tile_pool(name="loop", bufs=2))

        def pt(shape, dtype=f32, tag=None):
            return cp.tile(shape, dtype, tag=tag, bufs=1, name=tag or "ptile")

        QSTGH = spool.tile([3, NQ], f16, tag="S", name="QSTGH")
        nc.sync.dma_start(QSTGH[:, :], q2[:, :])
        QSTG = spool.tile([4, NQ], f32, tag="S", name="QSTG")
        nc.vector.memset(QSTG[:, :], -1.0)
        nc.vector.tensor_scalar(QSTG[0:3, :], QSTGH[:], 2.0, None, MULT)
        QAUG = pt([4, NQ], f32r, tag="QAUG")
        nc.gpsimd.tensor_copy(QAUG[:], QSTG[:])

        KBH = pt([32, 768], f16, tag="KBH")
        nc.sync.dma_start(KBH[:], kb[:, :])
        KBLK = pt([32, 768], tag="KBLK")
        nc.scalar.copy(KBLK[:], KBH[:])

        # [3, N] database layout rebuilt from the blocked form via
        # partition-collapse DMAs (32p x 256 -> 1p x 8192)
        KSTG = spool.tile([4, N], f32, tag="S", name="KSTG")
        for c in range(3):
            nc.sync.dma_start(KSTG[c:c + 1, :], KBLK[:, 256 * c:256 * (c + 1)])

        KSQ = pt([32, 768], tag="KSQ")
        nc.scalar.square(KSQ[:], KBLK[:])
        NORM = pt([32, 256], tag="NORM")
        nc.vector.tensor_tensor(NORM[:], KSQ[:, 0:256], KSQ[:, 256:512], ADD)
        nc.vector.tensor_tensor(NORM[:], NORM[:], KSQ[:, 512:768], ADD)
        nc.sync.dma_start(KSTG[3:4, :], NORM[:])
        KAUG = pt([4, N], f32r, tag="KAUG")
        nc.gpsimd.tensor_copy(KAUG[:], KSTG[:])

        VAL8 = pt([128, 8 * NT], f16, tag="VAL8")    # top-8 fp16 scores
        GIDX8 = pt([128, 8 * NT], i16, tag="GIDX8")  # top-8 indices

        for t in range(NT):
            S = spool.tile([128, N], f16, tag="S", name="S")
            lhsT = QAUG[:, bass.ts(t, 128)]
            for kc in range(4):
                P = ppool.tile([128, 2048], f32, tag="P", name="P")
                for i in range(4):
                    nc.tensor.matmul(
                        P[:, bass.ts(i, 512)],
                        lhsT,
                        KAUG[:, 2048 * kc + 512 * i:2048 * kc + 512 * (i + 1)],
                        start=True, stop=True)
                nc.scalar.copy(S[:, bass.ts(kc, 2048)], P[:])
            V8 = VAL8[:, 8 * t:8 * t + 8]
            nc.vector.max(V8, S[:])
            I8 = tp.tile([128, 8], u32, tag="I8", name="I8")
            nc.vector.max_index(I8[:], V8, S[:])
            nc.gpsimd.tensor_copy(GIDX8[:, 8 * t:8 * t + 8], I8[:])

        # ship only ranks 2 and 7 of each tile's top-8 (tie-flag inputs)
        V8R = VAL8.rearrange("p (t k) -> p t k", k=8)
        nc.sync.dma_start(vallo[:, 0:NT], V8R[:, :, 2])
        nc.sync.dma_start(vallo[:, NT:2 * NT], V8R[:, :, 7])
        nc.sync.dma_start(gidxo[:, :], GIDX8[:])

    nc.compile()
    _CACHE["nc"] = nc
    return nc


def _get_runner():
    """Build the 8-core shard_map executable once; return (run, dbg_name)."""
    if "runner" in _CACHE:
        return _CACHE["runner"]

    import jax
    import jax.core
    from jax.experimental.shard_map import shard_map
    from jax.sharding import Mesh, PartitionSpec
    from concourse import bass2jax, mybir

    nc = _build()
    bass2jax.install_neuronx_cc_hook()

    dbg_name = None
    if getattr(nc, "dbg_addr", None) is not None:
        if nc.dbg_callbacks:
            raise RuntimeError("dbg_callbacks unsupported under axon")
        dbg_name = nc.dbg_addr.name
    partition_name = (nc.partition_id_tensor.name
                      if nc.partition_id_tensor else None)

    in_names, out_names, out_avals = [], [], []
    for alloc in nc.m.functions[0].allocations:
        if not isinstance(alloc, mybir.MemoryLocationSet):
            continue
        name = alloc.memorylocations[0].name
        if alloc.kind == "ExternalInput":
            if name != partition_name:
                in_names.append(name)
        elif alloc.kind == "ExternalOutput":
            out_names.append(name)
            out_avals.append(jax.core.ShapedArray(
                tuple(alloc.tensor_shape), mybir.dt.np(alloc.dtype)))
    # the kernel writes every element of every output, so no pre-zeroed
    # donated output operands are needed — results are plain custom-call
    # outputs allocated by the runtime
    bind_in_names = list(in_names)
    if partition_name is not None:
        bind_in_names.append(partition_name)

    def _body(*args):
        operands = list(args)
        if partition_name is not None:
            operands.append(bass2jax.partition_id_tensor())
        outs = bass2jax._bass_exec_p.bind(
            *operands,
            out_avals=tuple(out_avals),
            in_names=tuple(bind_in_names),
            out_names=tuple(out_names),
            lowering_input_output_aliases=(),
            sim_require_finite=True,
            sim_require_nnan=True,
            nc=nc,
        )
        return tuple(outs)

    devices = jax.devices()[:8]
    mesh = Mesh(np.asarray(devices), ("core",))
    in_specs = (PartitionSpec("core"),) * len(in_names)
    out_specs = (PartitionSpec("core"),) * len(out_names)
    sharded = jax.jit(
        shard_map(_body, mesh=mesh, in_specs=in_specs,
                  out_specs=out_specs, check_rep=False),
        keep_unused=True,
    )

    def dispatch(concat_inputs):
        outs = sharded(*[concat_inputs[n] for n in in_names])
        # start all D2H copies before the first blocking asarray so the
        # fetches pipeline into a single axon roundtrip
        for o in outs:
            o.copy_to_host_async()
        return outs

    def fetch(outs):
        return {name: np.asarray(o) for name, o in zip(out_names, outs)}

    _CACHE["parts"] = (sharded, list(in_names), list(out_names), mesh)
    _CACHE["runner"] = (dispatch, fetch, dbg_name)
    return _CACHE["runner"]


def _prep_host(warped, pos2, flow1):
    """Device-independent combine inputs; runs while the fetch roundtrip
    is in flight."""
    # queries q[core, p, t, c] = pos2[b, c, h*4096 + t*128 + p]
    q = np.ascontiguousarray(
        pos2.reshape(B, C, 2, NT, 128).transpose(0, 2, 4, 3, 1)
    ).reshape(8, 128, NT, C)
    q2s = np.einsum('cptd,cptd->cpt', q, q)

    # 16-byte rows [kx, ky, kz, |k|^2] viewed as complex128 for fast
    # single-element gathers
    kpad = np.empty((B, N, 4), np.float32)
    kpad[:, :, :3] = warped.transpose(0, 2, 1)
    kpad[:, :, 3] = np.einsum('bnd,bnd->bn', kpad[..., :3], kpad[..., :3])
    kc128 = kpad.reshape(B * N, 4).view(np.complex128).reshape(B * N)
    fpad = np.zeros((B, N, 4), np.float32)
    fpad[:, :, :3] = flow1.transpose(0, 2, 1)
    fc128 = fpad.reshape(B * N, 4).view(np.complex128).reshape(B * N)
    return q, q2s, kpad, kc128, fc128


def _combine_all(prep, val_all, gidx_all):
    """Exact re-rank of device top-8 candidates + weighted warp, all cores.

    val_all: [8*128, 64] f16 (cols 0:32 = rank-2 value per tile, 32:64 =
    rank-7); gidx_all: [8*128, 256] i16.  Core c = 2b+h covers
    pos2[b,:,h*NQ:(h+1)*NQ]; device query (t,p) -> row p, cols 8t..8t+7.
    Returns the full [B, C, N] output.
    """
    q, q2s, kpad, kc128, fc128 = prep
    vv = np.asarray(val_all).reshape(8, 128, 2, NT)
    idx16 = np.asarray(gidx_all).reshape(8, 128, NT, 8)
    idx = idx16.astype(np.int32)

    boff = (np.arange(8, dtype=np.int32) // 2 * N)[:, None, None, None]
    gidx = idx + boff                                        # [8,128,NT,8]
    gk = kc128[gidx].view(np.float32).reshape(8, 128, NT, 8, 4)
    # d2 in the reference's dot form: |q|^2 - 2 q.k + |k|^2
    dot = np.einsum('cptkd,cptd->cptk', gk[..., :3], q,
                    dtype=np.float32, casting='unsafe')
    d2c = gk[..., 3] - 2.0 * dot + q2s[..., None]            # [8,128,NT,8]

    # exact hierarchical sort key: (f32 d2 bits << 13) | idx.  d2 >= 0 so
    # its bit pattern is order-preserving as an unsigned int.
    key = (d2c.view(np.int32).astype(np.int64) << 13) | idx.astype(np.int64)
    key3 = np.sort(key, axis=-1)[..., :3]
    i3 = (key3 & (N - 1)).astype(np.int32)
    d2_3 = (key3 >> 13).astype(np.int32).view(np.float32)

    # fp16 v3 == v8  =>  candidate set may be incomplete; dup indices too.
    # dups only arise from tied fp16 values, which are adjacent in the
    # sorted top-8, so an adjacency check is exact.
    flag = vv[:, :, 0, :] == vv[:, :, 1, :]
    flag |= (idx16[..., 1:] == idx16[..., :-1]).any(-1)
    if flag.any():
        fc, fp, ft = np.nonzero(flag)
        qf = q[fc, fp, ft]                                   # [m, 3]
        base = (fc // 2) * N
        kall = kpad[..., :3].reshape(B * N, 3)
        for j in range(len(fc)):
            kb = kall[base[j]:base[j] + N]
            d2f = ((qf[j][None, :] - kb) ** 2).sum(-1, dtype=np.float32)
            kf = (d2f.view(np.int32).astype(np.int64) << 13) \
                | np.arange(N, dtype=np.int64)
            k3 = np.sort(kf)[:3]
            i3[fc[j], fp[j], ft[j]] = (k3 & (N - 1)).astype(np.int32)
            d2_3[fc[j], fp[j], ft[j]] = \
                (k3 >> 13).astype(np.int32).view(np.float32)

    dist = np.maximum(np.sqrt(np.maximum(d2_3, 0.0)), EPS)
    inv = 1.0 / dist
    w = inv / inv.sum(-1, keepdims=True)                     # [8,128,NT,3]
    gfl = fc128[i3 + boff].view(np.float32).reshape(
        8, 128, NT, 3, 4)[..., :3]
    flow2 = np.einsum('cptk,cptkd->cptd', w, gfl)            # [8,128,NT,3]
    res = q - flow2
    np.clip(res, -CLAMP, CLAMP, out=res)
    # res[2b+h, p, t, c] -> full[b, c, h*4096 + t*128 + p]
    return np.ascontiguousarray(
        res.reshape(B, 2, 128, NT, C).transpose(0, 4, 1, 3, 2)
    ).reshape(B, C, N)


def kernel(pos1, pos2, flow1):
    pos1 = np.ascontiguousarray(np.asarray(pos1, dtype=np.float32))
    pos2 = np.ascontiguousarray(np.asarray(pos2, dtype=np.float32))
    flow1 = np.ascontiguousarray(np.asarray(flow1, dtype=np.float32))

    dispatch, fetch, dbg_name = _get_runner()
    warped = pos1 + flow1

    # per-core [3, NQ] query slabs, concatenated on axis 0 (f16 upload —
    # selection only; the exact host re-rank uses the f32 originals)
    q2_all = pos2.reshape(B, C, 2, NQ).transpose(0, 2, 1, 3).astype(
        np.float16).reshape(8 * C, NQ)
    # blocked [32, 768] pre-warped database k = pos1+flow1, repeated for
    # both query halves
    kb_all = np.repeat(
        warped.reshape(B, C, 32, 256).transpose(0, 2, 1, 3).astype(
            np.float16).reshape(B, 32, 768), 2, axis=0).reshape(8 * 32, 768)

    concat_inputs = {"q2": q2_all, "kb": kb_all}
    if dbg_name is not None:
        concat_inputs[dbg_name] = np.zeros((8, 2), np.uint32)

    outs = dispatch(concat_inputs)
    prep = _prep_host(warped, pos2, flow1)   # overlaps the roundtrip
    vals = fetch(outs)
    return _combine_all(prep, vals["vallo"], vals["gidxo"])


# revision 23
# speedup vs baseline: 1.0683x; 1.0683x over previous
"""PointWarping v4: fp16 score selection on device + exact host re-rank.

Device per core: augmented matmul scores (f32 PSUM) are cast to fp16 on
the PSUM->SBUF copy; DVE max / max_index run at 2x 16-bit throughput and
return the top-8 candidate values+indices per query.  Host re-ranks the
8 candidates with exact f32 distances (reference formula), computes the
weights, gathers neighbor flows and warps.  Queries where the fp16
3rd==8th value ties (candidate set not provably complete) or duplicate
indices appear are recomputed exactly on host (rare).

Perf notes (axon-tunneled cores: ~80ms RTT, ~170MB/s, ~6ms/exec launch):
- the PJRT shard_map executable is built once and cached
- no donated zero output buffers (kernel writes every output element),
  so nothing but the real inputs is uploaded per call
- all D2H copies start async so both outputs fetch in one roundtrip
- vallo ships only the rank-2 and rank-7 values (the tie flag inputs)
- host combine is vectorized over all 8 cores; 16-byte-row gathers go
  through a complex128 view (single-element fancy indexing)
"""

import numpy as np

B, C, N = 4, 3, 8192
NQ = 4096
NT = 32
EPS = 1e-10
CLAMP = 10.0

_CACHE = {}


def _build():
    if "nc" in _CACHE:
        return _CACHE["nc"]

    from contextlib import ExitStack
    from concourse import bacc, bass, tile
    from concourse import mybir

    nc = bacc.Bacc("TRN2", target_bir_lowering=False, debug=False,
                   enable_asserts=True, num_devices=1)
    f32 = mybir.dt.float32
    f32r = mybir.dt.float32r
    f16 = mybir.dt.float16
    i16 = mybir.dt.int16
    u32 = mybir.dt.uint32
    ADD = mybir.AluOpType.add
    MULT = mybir.AluOpType.mult

    # packed input: rows 0:16 = queries ([3,4096] f16 flat), 16:48 = the
    # blocked pre-warped database; packed output: cols 0:256 = top-8
    # indices, 256:320 = fp16 rank-2/rank-7 score bits
    pk = nc.dram_tensor("pk", [48, 768], f16, kind="ExternalInput").ap()
    outo = nc.dram_tensor("outo", [128, 8 * NT + 2 * NT], i16,
                          kind="ExternalOutput").ap()

    with tile.TileContext(nc) as tc, ExitStack() as ctx:
        cp = ctx.enter_context(tc.tile_pool(name="persist", bufs=1))
        spool = ctx.enter_context(tc.tile_pool(name="scores", bufs=2))
        ppool = ctx.enter_context(tc.tile_pool(name="ps", bufs=2, space="PSUM"))
        tp = ctx.enter_context(tc.tile_pool(name="loop", bufs=2))

        def pt(shape, dtype=f32, tag=None):
            return cp.tile(shape, dtype, tag=tag, bufs=1, name=tag or "ptile")

        QSTGH = spool.tile([3, NQ], f16, tag="S", name="QSTGH")
        nc.sync.dma_start(QSTGH[:, :], pk[0:16, :])
        QSTG = spool.tile([4, NQ], f32, tag="S", name="QSTG")
        nc.vector.memset(QSTG[:, :], -1.0)
        nc.vector.tensor_scalar(QSTG[0:3, :], QSTGH[:], 2.0, None, MULT)
        QAUG = pt([4, NQ], f32r, tag="QAUG")
        nc.gpsimd.tensor_copy(QAUG[:], QSTG[:])

        KBH = pt([32, 768], f16, tag="KBH")
        nc.sync.dma_start(KBH[:], pk[16:48, :])
        KBLK = pt([32, 768], tag="KBLK")
        nc.scalar.copy(KBLK[:], KBH[:])

        # [3, N] database layout rebuilt from the blocked form via
        # partition-collapse DMAs (32p x 256 -> 1p x 8192)
        KSTG = spool.tile([4, N], f32, tag="S", name="KSTG")
        for c in range(3):
            nc.sync.dma_start(KSTG[c:c + 1, :], KBLK[:, 256 * c:256 * (c + 1)])

        KSQ = pt([32, 768], tag="KSQ")
        nc.scalar.square(KSQ[:], KBLK[:])
        NORM = pt([32, 256], tag="NORM")
        nc.vector.tensor_tensor(NORM[:], KSQ[:, 0:256], KSQ[:, 256:512], ADD)
        nc.vector.tensor_tensor(NORM[:], NORM[:], KSQ[:, 512:768], ADD)
        nc.sync.dma_start(KSTG[3:4, :], NORM[:])
        KAUG = pt([4, N], f32r, tag="KAUG")
        nc.gpsimd.tensor_copy(KAUG[:], KSTG[:])

        VAL8 = pt([128, 8 * NT], f16, tag="VAL8")    # top-8 fp16 scores
        GIDX8 = pt([128, 8 * NT], i16, tag="GIDX8")  # top-8 indices

        for t in range(NT):
            S = spool.tile([128, N], f16, tag="S", name="S")
            lhsT = QAUG[:, bass.ts(t, 128)]
            for kc in range(4):
                P = ppool.tile([128, 2048], f32, tag="P", name="P")
                for i in range(4):
                    nc.tensor.matmul(
                        P[:, bass.ts(i, 512)],
                        lhsT,
                        KAUG[:, 2048 * kc + 512 * i:2048 * kc + 512 * (i + 1)],
                        start=True, stop=True)
                nc.scalar.copy(S[:, bass.ts(kc, 2048)], P[:])
            V8 = VAL8[:, 8 * t:8 * t + 8]
            nc.vector.max(V8, S[:])
            I8 = tp.tile([128, 8], u32, tag="I8", name="I8")
            nc.vector.max_index(I8[:], V8, S[:])
            nc.gpsimd.tensor_copy(GIDX8[:, 8 * t:8 * t + 8], I8[:])

        # ship the indices plus ranks 2 and 7 of each tile's top-8 (the
        # tie-flag inputs), packed into one output tensor
        V8R = VAL8.rearrange("p (t k) -> p t k", k=8)
        nc.sync.dma_start(outo[:, 0:8 * NT], GIDX8[:])
        nc.sync.dma_start(outo[:, 8 * NT:9 * NT], V8R[:, :, 2].bitcast(i16))
        nc.sync.dma_start(outo[:, 9 * NT:10 * NT], V8R[:, :, 7].bitcast(i16))

    nc.compile()
    _CACHE["nc"] = nc
    return nc


def _get_runner():
    """Build the 8-core shard_map executable once; return (run, dbg_name)."""
    if "runner" in _CACHE:
        return _CACHE["runner"]

    import jax
    import jax.core
    from jax.experimental.shard_map import shard_map
    from jax.sharding import Mesh, PartitionSpec
    from concourse import bass2jax, mybir

    nc = _build()
    bass2jax.install_neuronx_cc_hook()

    dbg_name = None
    if getattr(nc, "dbg_addr", None) is not None:
        if nc.dbg_callbacks:
            raise RuntimeError("dbg_callbacks unsupported under axon")
        dbg_name = nc.dbg_addr.name
    partition_name = (nc.partition_id_tensor.name
                      if nc.partition_id_tensor else None)

    in_names, out_names, out_avals = [], [], []
    for alloc in nc.m.functions[0].allocations:
        if not isinstance(alloc, mybir.MemoryLocationSet):
            continue
        name = alloc.memorylocations[0].name
        if alloc.kind == "ExternalInput":
            if name != partition_name:
                in_names.append(name)
        elif alloc.kind == "ExternalOutput":
            out_names.append(name)
            out_avals.append(jax.core.ShapedArray(
                tuple(alloc.tensor_shape), mybir.dt.np(alloc.dtype)))
    # the kernel writes every element of every output, so no pre-zeroed
    # donated output operands are needed — results are plain custom-call
    # outputs allocated by the runtime
    bind_in_names = list(in_names)
    if partition_name is not None:
        bind_in_names.append(partition_name)

    def _body(*args):
        operands = list(args)
        if partition_name is not None:
            operands.append(bass2jax.partition_id_tensor())
        outs = bass2jax._bass_exec_p.bind(
            *operands,
            out_avals=tuple(out_avals),
            in_names=tuple(bind_in_names),
            out_names=tuple(out_names),
            lowering_input_output_aliases=(),
            sim_require_finite=True,
            sim_require_nnan=True,
            nc=nc,
        )
        return tuple(outs)

    devices = jax.devices()[:8]
    mesh = Mesh(np.asarray(devices), ("core",))
    in_specs = (PartitionSpec("core"),) * len(in_names)
    out_specs = (PartitionSpec("core"),) * len(out_names)
    sharded = jax.jit(
        shard_map(_body, mesh=mesh, in_specs=in_specs,
                  out_specs=out_specs, check_rep=False),
        keep_unused=True,
    )

    def dispatch(concat_inputs):
        outs = sharded(*[concat_inputs[n] for n in in_names])
        # start all D2H copies before the first blocking asarray so the
        # fetches pipeline into a single axon roundtrip
        for o in outs:
            o.copy_to_host_async()
        return outs

    def fetch(outs):
        return {name: np.asarray(o) for name, o in zip(out_names, outs)}

    _CACHE["parts"] = (sharded, list(in_names), list(out_names), mesh)
    _CACHE["runner"] = (dispatch, fetch, dbg_name)
    return _CACHE["runner"]


def _prep_host(warped, pos2, flow1):
    """Device-independent combine inputs; runs while the fetch roundtrip
    is in flight."""
    # queries q[core, p, t, c] = pos2[b, c, h*4096 + t*128 + p]
    q = np.ascontiguousarray(
        pos2.reshape(B, C, 2, NT, 128).transpose(0, 2, 4, 3, 1)
    ).reshape(8, 128, NT, C)
    q2s = np.einsum('cptd,cptd->cpt', q, q)

    # 16-byte rows [kx, ky, kz, |k|^2] viewed as complex128 for fast
    # single-element gathers
    kpad = np.empty((B, N, 4), np.float32)
    kpad[:, :, :3] = warped.transpose(0, 2, 1)
    kpad[:, :, 3] = np.einsum('bnd,bnd->bn', kpad[..., :3], kpad[..., :3])
    kc128 = kpad.reshape(B * N, 4).view(np.complex128).reshape(B * N)
    fpad = np.zeros((B, N, 4), np.float32)
    fpad[:, :, :3] = flow1.transpose(0, 2, 1)
    fc128 = fpad.reshape(B * N, 4).view(np.complex128).reshape(B * N)
    return q, q2s, kpad, kc128, fc128


def _combine_all(prep, val_all, gidx_all):
    """Exact re-rank of device top-8 candidates + weighted warp, all cores.

    val_all: [8*128, 64] f16 (cols 0:32 = rank-2 value per tile, 32:64 =
    rank-7); gidx_all: [8*128, 256] i16.  Core c = 2b+h covers
    pos2[b,:,h*NQ:(h+1)*NQ]; device query (t,p) -> row p, cols 8t..8t+7.
    Returns the full [B, C, N] output.
    """
    q, q2s, kpad, kc128, fc128 = prep
    vv = np.asarray(val_all).reshape(8, 128, 2, NT)
    idx16 = np.asarray(gidx_all).reshape(8, 128, NT, 8)
    idx = idx16.astype(np.int32)

    boff = (np.arange(8, dtype=np.int32) // 2 * N)[:, None, None, None]
    gidx = idx + boff                                        # [8,128,NT,8]
    gk = kc128[gidx].view(np.float32).reshape(8, 128, NT, 8, 4)
    # d2 in the reference's dot form: |q|^2 - 2 q.k + |k|^2
    dot = np.einsum('cptkd,cptd->cptk', gk[..., :3], q,
                    dtype=np.float32, casting='unsafe')
    d2c = gk[..., 3] - 2.0 * dot + q2s[..., None]            # [8,128,NT,8]

    # exact hierarchical sort key: (f32 d2 bits << 13) | idx.  d2 >= 0 so
    # its bit pattern is order-preserving as an unsigned int.
    key = (d2c.view(np.int32).astype(np.int64) << 13) | idx.astype(np.int64)
    key3 = np.sort(key, axis=-1)[..., :3]
    i3 = (key3 & (N - 1)).astype(np.int32)
    d2_3 = (key3 >> 13).astype(np.int32).view(np.float32)

    # fp16 v3 == v8  =>  candidate set may be incomplete; dup indices too.
    # dups only arise from tied fp16 values, which are adjacent in the
    # sorted top-8, so an adjacency check is exact.
    flag = vv[:, :, 0, :] == vv[:, :, 1, :]
    flag |= (idx16[..., 1:] == idx16[..., :-1]).any(-1)
    if flag.any():
        fc, fp, ft = np.nonzero(flag)
        qf = q[fc, fp, ft]                                   # [m, 3]
        base = (fc // 2) * N
        kall = kpad[..., :3].reshape(B * N, 3)
        for j in range(len(fc)):
            kb = kall[base[j]:base[j] + N]
            d2f = ((qf[j][None, :] - kb) ** 2).sum(-1, dtype=np.float32)
            kf = (d2f.view(np.int32).astype(np.int64) << 13) \
                | np.arange(N, dtype=np.int64)
            k3 = np.sort(kf)[:3]
            i3[fc[j], fp[j], ft[j]] = (k3 & (N - 1)).astype(np.int32)
            d2_3[fc[j], fp[j], ft[j]] = \
                (k3 >> 13).astype(np.int32).view(np.float32)

    dist = np.maximum(np.sqrt(np.maximum(d2_3, 0.0)), EPS)
    inv = 1.0 / dist
    w = inv / inv.sum(-1, keepdims=True)                     # [8,128,NT,3]
    gfl = fc128[i3 + boff].view(np.float32).reshape(
        8, 128, NT, 3, 4)[..., :3]
    flow2 = np.einsum('cptk,cptkd->cptd', w, gfl)            # [8,128,NT,3]
    res = q - flow2
    np.clip(res, -CLAMP, CLAMP, out=res)
    # res[2b+h, p, t, c] -> full[b, c, h*4096 + t*128 + p]
    return np.ascontiguousarray(
        res.reshape(B, 2, 128, NT, C).transpose(0, 4, 1, 3, 2)
    ).reshape(B, C, N)


def kernel(pos1, pos2, flow1):
    pos1 = np.ascontiguousarray(np.asarray(pos1, dtype=np.float32))
    pos2 = np.ascontiguousarray(np.asarray(pos2, dtype=np.float32))
    flow1 = np.ascontiguousarray(np.asarray(flow1, dtype=np.float32))

    dispatch, fetch, dbg_name = _get_runner()
    warped = pos1 + flow1

    # packed per-core input: rows 0:16 query slab ([3,4096] f16 flat),
    # rows 16:48 blocked pre-warped database (f16 upload — selection
    # only; the exact host re-rank uses the f32 originals)
    pk = np.empty((8, 48, 768), np.float16)
    pk[:, 0:16] = pos2.reshape(B, C, 2, NQ).transpose(0, 2, 1, 3).astype(
        np.float16).reshape(8, 16, 768)
    pk[:, 16:48] = warped.reshape(B, C, 32, 256).transpose(
        0, 2, 1, 3).astype(np.float16).reshape(B, 32, 768)[
        np.arange(8) // 2]

    concat_inputs = {"pk": pk.reshape(8 * 48, 768)}
    if dbg_name is not None:
        concat_inputs[dbg_name] = np.zeros((8, 2), np.uint32)

    outs = dispatch(concat_inputs)
    prep = _prep_host(warped, pos2, flow1)   # overlaps the roundtrip
    vals = fetch(outs)
    out = vals["outo"]
    gidx_all = out[:, 0:8 * NT]
    val_all = np.ascontiguousarray(out[:, 8 * NT:10 * NT]).view(np.float16)
    return _combine_all(prep, val_all, gidx_all)


# revision 26
# speedup vs baseline: 1.0994x; 1.0292x over previous
"""PointWarping v4: fp16 score selection on device + exact host re-rank.

Device per core: augmented matmul scores (f32 PSUM) are cast to fp16 on
the PSUM->SBUF copy; DVE max / max_index run at 2x 16-bit throughput and
return the top-8 candidate values+indices per query.  Host re-ranks the
8 candidates with exact f32 distances (reference formula), computes the
weights, gathers neighbor flows and warps.  Queries where the fp16
3rd==8th value ties (candidate set not provably complete) or duplicate
indices appear are recomputed exactly on host (rare).

Perf notes (axon-tunneled cores: ~80ms RTT, ~170MB/s, ~6ms/exec launch):
- the PJRT shard_map executable is built once and cached
- no donated zero output buffers (kernel writes every output element),
  so nothing but the real inputs is uploaded per call
- all D2H copies start async so both outputs fetch in one roundtrip
- vallo ships only the rank-2 and rank-7 values (the tie flag inputs)
- host combine is vectorized over all 8 cores; 16-byte-row gathers go
  through a complex128 view (single-element fancy indexing)
"""

import numpy as np

B, C, N = 4, 3, 8192
NQ = 4096
NT = 32
EPS = 1e-10
CLAMP = 10.0

_CACHE = {}


def _build():
    if "nc" in _CACHE:
        return _CACHE["nc"]

    from contextlib import ExitStack
    from concourse import bacc, bass, tile
    from concourse import mybir

    nc = bacc.Bacc("TRN2", target_bir_lowering=False, debug=False,
                   enable_asserts=True, num_devices=1)
    f32 = mybir.dt.float32
    f32r = mybir.dt.float32r
    f16 = mybir.dt.float16
    i16 = mybir.dt.int16
    u32 = mybir.dt.uint32
    ADD = mybir.AluOpType.add
    MULT = mybir.AluOpType.mult

    # packed input: rows 0:16 = queries ([3,4096] f16 flat), 16:48 = the
    # blocked pre-warped database; packed output: cols 0:256 = top-8
    # indices, 256:320 = fp16 rank-2/rank-7 score bits
    pk = nc.dram_tensor("pk", [48, 768], f16, kind="ExternalInput").ap()
    outo = nc.dram_tensor("outo", [128, 8 * NT + 2 * NT], i16,
                          kind="ExternalOutput").ap()

    with tile.TileContext(nc) as tc, ExitStack() as ctx:
        cp = ctx.enter_context(tc.tile_pool(name="persist", bufs=1))
        spool = ctx.enter_context(tc.tile_pool(name="scores", bufs=2))
        ppool = ctx.enter_context(tc.tile_pool(name="ps", bufs=2, space="PSUM"))
        tp = ctx.enter_context(tc.tile_pool(name="loop", bufs=2))

        def pt(shape, dtype=f32, tag=None):
            return cp.tile(shape, dtype, tag=tag, bufs=1, name=tag or "ptile")

        QSTGH = spool.tile([3, NQ], f16, tag="S", name="QSTGH")
        nc.sync.dma_start(QSTGH[:, :], pk[0:16, :])
        QSTG = spool.tile([4, NQ], f32, tag="S", name="QSTG")
        nc.vector.memset(QSTG[:, :], -1.0)
        nc.vector.tensor_scalar(QSTG[0:3, :], QSTGH[:], 2.0, None, MULT)
        QAUG = pt([4, NQ], f32r, tag="QAUG")
        nc.gpsimd.tensor_copy(QAUG[:], QSTG[:])

        KBH = pt([32, 768], f16, tag="KBH")
        nc.sync.dma_start(KBH[:], pk[16:48, :])
        KBLK = pt([32, 768], tag="KBLK")
        nc.scalar.copy(KBLK[:], KBH[:])

        # [3, N] database layout rebuilt from the blocked form via
        # partition-collapse DMAs (32p x 256 -> 1p x 8192)
        KSTG = spool.tile([4, N], f32, tag="S", name="KSTG")
        for c in range(3):
            nc.sync.dma_start(KSTG[c:c + 1, :], KBLK[:, 256 * c:256 * (c + 1)])

        KSQ = pt([32, 768], tag="KSQ")
        nc.scalar.square(KSQ[:], KBLK[:])
        NORM = pt([32, 256], tag="NORM")
        nc.vector.tensor_tensor(NORM[:], KSQ[:, 0:256], KSQ[:, 256:512], ADD)
        nc.vector.tensor_tensor(NORM[:], NORM[:], KSQ[:, 512:768], ADD)
        nc.sync.dma_start(KSTG[3:4, :], NORM[:])
        KAUG = pt([4, N], f32r, tag="KAUG")
        nc.gpsimd.tensor_copy(KAUG[:], KSTG[:])

        VAL8 = pt([128, 8 * NT], f16, tag="VAL8")    # top-8 fp16 scores
        GIDX8 = pt([128, 8 * NT], i16, tag="GIDX8")  # top-8 indices

        for t in range(NT):
            S = spool.tile([128, N], f16, tag="S", name="S")
            lhsT = QAUG[:, bass.ts(t, 128)]
            for kc in range(4):
                P = ppool.tile([128, 2048], f32, tag="P", name="P")
                for i in range(4):
                    nc.tensor.matmul(
                        P[:, bass.ts(i, 512)],
                        lhsT,
                        KAUG[:, 2048 * kc + 512 * i:2048 * kc + 512 * (i + 1)],
                        start=True, stop=True)
                nc.scalar.copy(S[:, bass.ts(kc, 2048)], P[:])
            V8 = VAL8[:, 8 * t:8 * t + 8]
            nc.vector.max(V8, S[:])
            I8 = tp.tile([128, 8], u32, tag="I8", name="I8")
            nc.vector.max_index(I8[:], V8, S[:])
            nc.gpsimd.tensor_copy(GIDX8[:, 8 * t:8 * t + 8], I8[:])

        # ship the indices plus ranks 2 and 7 of each tile's top-8 (the
        # tie-flag inputs), packed into one output tensor
        V8R = VAL8.rearrange("p (t k) -> p t k", k=8)
        nc.sync.dma_start(outo[:, 0:8 * NT], GIDX8[:])
        nc.sync.dma_start(outo[:, 8 * NT:9 * NT], V8R[:, :, 2].bitcast(i16))
        nc.sync.dma_start(outo[:, 9 * NT:10 * NT], V8R[:, :, 7].bitcast(i16))

    nc.compile()
    _CACHE["nc"] = nc
    return nc


def _get_runner():
    """Build the 8-core shard_map executable once; return (run, dbg_name)."""
    if "runner" in _CACHE:
        return _CACHE["runner"]

    import jax
    import jax.core
    from jax.experimental.shard_map import shard_map
    from jax.sharding import Mesh, PartitionSpec
    from concourse import bass2jax, mybir

    nc = _build()
    bass2jax.install_neuronx_cc_hook()

    dbg_name = None
    if getattr(nc, "dbg_addr", None) is not None:
        if nc.dbg_callbacks:
            raise RuntimeError("dbg_callbacks unsupported under axon")
        dbg_name = nc.dbg_addr.name
    partition_name = (nc.partition_id_tensor.name
                      if nc.partition_id_tensor else None)

    in_names, out_names, out_avals = [], [], []
    for alloc in nc.m.functions[0].allocations:
        if not isinstance(alloc, mybir.MemoryLocationSet):
            continue
        name = alloc.memorylocations[0].name
        if alloc.kind == "ExternalInput":
            if name != partition_name:
                in_names.append(name)
        elif alloc.kind == "ExternalOutput":
            out_names.append(name)
            out_avals.append(jax.core.ShapedArray(
                tuple(alloc.tensor_shape), mybir.dt.np(alloc.dtype)))
    # the kernel writes every element of every output, so no pre-zeroed
    # donated output operands are needed — results are plain custom-call
    # outputs allocated by the runtime
    bind_in_names = list(in_names)
    if partition_name is not None:
        bind_in_names.append(partition_name)

    def _body(*args):
        operands = list(args)
        if partition_name is not None:
            operands.append(bass2jax.partition_id_tensor())
        outs = bass2jax._bass_exec_p.bind(
            *operands,
            out_avals=tuple(out_avals),
            in_names=tuple(bind_in_names),
            out_names=tuple(out_names),
            lowering_input_output_aliases=(),
            sim_require_finite=True,
            sim_require_nnan=True,
            nc=nc,
        )
        return tuple(outs)

    devices = jax.devices()[:8]
    mesh = Mesh(np.asarray(devices), ("core",))
    in_specs = (PartitionSpec("core"),) * len(in_names)
    out_specs = (PartitionSpec("core"),) * len(out_names)
    sharded = jax.jit(
        shard_map(_body, mesh=mesh, in_specs=in_specs,
                  out_specs=out_specs, check_rep=False),
        keep_unused=True,
    )

    def dispatch(concat_inputs):
        outs = sharded(*[concat_inputs[n] for n in in_names])
        # start all D2H copies before the first blocking asarray so the
        # fetches pipeline into a single axon roundtrip
        for o in outs:
            o.copy_to_host_async()
        return outs

    def fetch(outs):
        return {name: np.asarray(o) for name, o in zip(out_names, outs)}

    _CACHE["parts"] = (sharded, list(in_names), list(out_names), mesh)
    _CACHE["runner"] = (dispatch, fetch, dbg_name)
    return _CACHE["runner"]


def _prep_host(warped, pos2, flow1):
    """Device-independent combine inputs; runs while the fetch roundtrip
    is in flight."""
    # queries q[core, p, t, c] = pos2[b, c, h*4096 + t*128 + p]
    q = np.ascontiguousarray(
        pos2.reshape(B, C, 2, NT, 128).transpose(0, 2, 4, 3, 1)
    ).reshape(8, 128, NT, C)
    q2s = np.einsum('cptd,cptd->cpt', q, q)
    # augmented query [-2q, 1]: dot with [k, |k|^2] rows gives |k|^2 - 2q.k
    q4 = np.empty((8, 128, NT, 4), np.float32)
    np.multiply(q, -2.0, out=q4[..., :3])
    q4[..., 3] = 1.0

    # 16-byte rows [kx, ky, kz, |k|^2] viewed as complex128 for fast
    # single-element gathers
    kpad = np.empty((B, N, 4), np.float32)
    kpad[:, :, :3] = warped.transpose(0, 2, 1)
    kpad[:, :, 3] = np.einsum('bnd,bnd->bn', kpad[..., :3], kpad[..., :3])
    kc128 = kpad.reshape(B * N, 4).view(np.complex128).reshape(B * N)
    fpad = np.zeros((B, N, 4), np.float32)
    fpad[:, :, :3] = flow1.transpose(0, 2, 1)
    fc128 = fpad.reshape(B * N, 4).view(np.complex128).reshape(B * N)
    return q, q2s, q4, kpad, kc128, fc128


def _combine_all(prep, val_all, gidx_all):
    """Exact re-rank of device top-8 candidates + weighted warp, all cores.

    val_all: [8*128, 64] f16 (cols 0:32 = rank-2 value per tile, 32:64 =
    rank-7); gidx_all: [8*128, 256] i16.  Core c = 2b+h covers
    pos2[b,:,h*NQ:(h+1)*NQ]; device query (t,p) -> row p, cols 8t..8t+7.
    Returns the full [B, C, N] output.
    """
    q, q2s, q4, kpad, kc128, fc128 = prep
    bufs = _CACHE.get("bufs")
    if bufs is None:
        bufs = {
            "gkc": np.empty((8, 128, NT, 8), np.complex128),
            "d2c": np.empty((8, 128, NT, 8), np.float32),
            "key": np.empty((8, 128, NT, 8), np.int64),
        }
        _CACHE["bufs"] = bufs
    vv = np.asarray(val_all).reshape(8, 128, 2, NT)
    idx16 = np.asarray(gidx_all).reshape(8, 128, NT, 8)
    idx = idx16.astype(np.int32)

    boff = (np.arange(8, dtype=np.int32) // 2 * N)[:, None, None, None]
    gidx = idx + boff                                        # [8,128,NT,8]
    gkc = bufs["gkc"]
    np.take(kc128, gidx, out=gkc)
    gk = gkc.view(np.float32).reshape(8, 128, NT, 8, 4)
    # d2 in the reference's dot form: |q|^2 + ([k,|k|^2] . [-2q, 1])
    d2c = np.einsum('cptkd,cptd->cptk', gk, q4, out=bufs["d2c"],
                    dtype=np.float32, casting='unsafe')
    d2c += q2s[..., None]                                    # [8,128,NT,8]

    # exact hierarchical sort key: (f32 d2 bits << 13) | idx.  d2 >= 0 so
    # its bit pattern is order-preserving as an unsigned int.
    key = bufs["key"]
    key[...] = d2c.view(np.int32)
    key <<= 13
    key |= idx
    key.sort(axis=-1)
    key3 = key[..., :3]
    i3 = (key3 & (N - 1)).astype(np.int32)
    d2_3 = (key3 >> 13).astype(np.int32).view(np.float32)

    # fp16 v3 == v8  =>  candidate set may be incomplete; dup indices too.
    # dups only arise from tied fp16 values, which are adjacent in the
    # sorted top-8, so an adjacency check is exact.
    flag = vv[:, :, 0, :] == vv[:, :, 1, :]
    flag |= (idx16[..., 1:] == idx16[..., :-1]).any(-1)
    if flag.any():
        fc, fp, ft = np.nonzero(flag)
        qf = q[fc, fp, ft]                                   # [m, 3]
        base = (fc // 2) * N
        kall = kpad[..., :3].reshape(B * N, 3)
        for j in range(len(fc)):
            kb = kall[base[j]:base[j] + N]
            d2f = ((qf[j][None, :] - kb) ** 2).sum(-1, dtype=np.float32)
            kf = (d2f.view(np.int32).astype(np.int64) << 13) \
                | np.arange(N, dtype=np.int64)
            k3 = np.sort(kf)[:3]
            i3[fc[j], fp[j], ft[j]] = (k3 & (N - 1)).astype(np.int32)
            d2_3[fc[j], fp[j], ft[j]] = \
                (k3 >> 13).astype(np.int32).view(np.float32)

    dist = np.maximum(np.sqrt(np.maximum(d2_3, 0.0)), EPS)
    inv = 1.0 / dist
    w = inv / inv.sum(-1, keepdims=True)                     # [8,128,NT,3]
    gfl = fc128[i3 + boff].view(np.float32).reshape(
        8, 128, NT, 3, 4)[..., :3]
    flow2 = np.einsum('cptk,cptkd->cptd', w, gfl)            # [8,128,NT,3]
    res = q - flow2
    np.clip(res, -CLAMP, CLAMP, out=res)
    # res[2b+h, p, t, c] -> full[b, c, h*4096 + t*128 + p]
    return np.ascontiguousarray(
        res.reshape(B, 2, 128, NT, C).transpose(0, 4, 1, 3, 2)
    ).reshape(B, C, N)


def kernel(pos1, pos2, flow1):
    pos1 = np.ascontiguousarray(np.asarray(pos1, dtype=np.float32))
    pos2 = np.ascontiguousarray(np.asarray(pos2, dtype=np.float32))
    flow1 = np.ascontiguousarray(np.asarray(flow1, dtype=np.float32))

    dispatch, fetch, dbg_name = _get_runner()
    warped = pos1 + flow1

    # packed per-core input: rows 0:16 query slab ([3,4096] f16 flat),
    # rows 16:48 blocked pre-warped database (f16 upload — selection
    # only; the exact host re-rank uses the f32 originals)
    pk = np.empty((8, 48, 768), np.float16)
    pk[:, 0:16] = pos2.reshape(B, C, 2, NQ).transpose(0, 2, 1, 3).astype(
        np.float16).reshape(8, 16, 768)
    pk[:, 16:48] = warped.reshape(B, C, 32, 256).transpose(
        0, 2, 1, 3).astype(np.float16).reshape(B, 32, 768)[
        np.arange(8) // 2]

    concat_inputs = {"pk": pk.reshape(8 * 48, 768)}
    if dbg_name is not None:
        concat_inputs[dbg_name] = np.zeros((8, 2), np.uint32)

    outs = dispatch(concat_inputs)
    prep = _prep_host(warped, pos2, flow1)   # overlaps the roundtrip
    vals = fetch(outs)
    out = vals["outo"]
    gidx_all = out[:, 0:8 * NT]
    val_all = np.ascontiguousarray(out[:, 8 * NT:10 * NT]).view(np.float16)
    return _combine_all(prep, val_all, gidx_all)
